# revision 93
# baseline (speedup 1.0000x reference)
"""Trainium2 Bass kernel for CausalCrossAttention (B=8, T=769, C=1024, H=16).

Sharding: data-parallel over batch B=8 across the 8 NeuronCores (one batch
element per core, SPMD — identical program, different input slices).

v2 — all-bf16 matmul operands (fp32 PSUM accumulation), designed around the
trace of the fp32r v1 (340 us):
  * bf16 stationary operands get Fast Weight Load (fp32 LDWEIGHTS was ~45% of
    PE busy time in v1) and bf16 moving operands avoid the fp32r 4-cycle/row
    penalty on free dims < 256 (the attention boundary tiles).
  * S^T per head-pair via K=64 row-tiled matmul pairs (tile_position rows 0/64
    run concurrently in the PE array) -> one N-stream per pair instead of one
    per head, and no sibling-row zeroing.
  * The softmax exp chain on the scalar engine (~76 us serial) is the #2
    resource after the PE, so S^T+exp "doses" are interleaved into the Q/K/V
    projection emission to start it ~7 us in instead of after projections.
  * Output projection in [c_out, t] layout (host un-transposes) -> same cost
    as Q/K and no M=1 tail streams.
  * PV PSUM slots are released by plain copies into yT right after the PV
    matmuls; the softmax divide (denominator row -> DRAM -> partition
    broadcast -> reciprocal -> in-place mul) runs off the critical path.

PSUM budget: psP 4x[128,512] (projections, PV, out-proj) + psS 2x[128,1024]
(S^T tiles, exp-consumed) = 8 banks.
"""

import os

import numpy as np

B, T, C = 8, 769, 1024
H, HD, L = 16, 64, 32
COND = 256
NCI = 8  # 1024 / 128 contraction tiles
NCO = 8
NTT = 7  # t tiles: 6 full + 1 single row
TP = 770  # streamed T padded to even
VW = H * (HD + 1) + 63  # V_aug free width, padded so M=128 slices stay in-bounds

# Per-(kv-tile) q ranges in the 0:512 block + mask offset.
# nk covers kv cols [128*nk, 128*nk+128); allowed iff kv_col < 256 + q_col,
# i.e. p < f + 256 - 128*nk with p the in-tile kv index, f the abs q col.
R0SUB = {0: (0, None), 1: (0, None), 2: (0, 0),
         3: (128, 128), 4: (256, 256), 5: (384, 384)}

_CACHE = {}


def _build_program():
    import concourse.mybir as mybir
    import concourse.tile as tile
    from concourse import bacc

    f32 = mybir.dt.float32
    bf = mybir.dt.bfloat16
    Exp = mybir.ActivationFunctionType.Exp
    Ident = mybir.ActivationFunctionType.Identity
    Copy = mybir.ActivationFunctionType.Copy

    nc = bacc.Bacc("TRN2", target_bir_lowering=False)

    # All big inputs are host-pre-tiled to the SBUF layout [partition, free...]
    # so each DMA moves one large contiguous chunk per partition (the DMA
    # queues are descriptor-rate bound, ~1 descriptor per (partition, chunk)).
    xqT_d = nc.dram_tensor("xqT3", [128, NCI, TP], bf, kind="ExternalInput")
    xkvT_d = nc.dram_tensor("xkvT3", [128, NCI, TP], bf, kind="ExternalInput")
    # Q/K weights co-major [p, co, ci, 128] so the first co pair's slice is
    # one small leading DMA; V/P weights ci-major [p, ci, co].
    wq_d = nc.dram_tensor("wqT3", [128, NCO, NCI, 128], bf,
                          kind="ExternalInput")
    wk_d = nc.dram_tensor("wkT3", [128, NCO, NCI, 128], bf,
                          kind="ExternalInput")
    wv_d = nc.dram_tensor("wvT3", [128, NCI, C], bf, kind="ExternalInput")
    wp_d = nc.dram_tensor("wpT3", [128, NCI, C], bf, kind="ExternalInput")
    bq_d = nc.dram_tensor("bq2", [128, NCO], f32, kind="ExternalInput")
    bk_d = nc.dram_tensor("bk2", [128, NCO], f32, kind="ExternalInput")
    bp_d = nc.dram_tensor("bp2", [128, NCO], f32, kind="ExternalInput")
    bv_d = nc.dram_tensor("bv1", [1, C], bf, kind="ExternalInput")
    cos_d = nc.dram_tensor("cosP", [128, TP], bf, kind="ExternalInput")
    sin_d = nc.dram_tensor("sinP", [128, TP], bf, kind="ExternalInput")
    m0_d = nc.dram_tensor("m0", [128, 128], bf, kind="ExternalInput")
    outT_d = nc.dram_tensor("outT", [C, TP], bf, kind="ExternalOutput")

    with tile.TileContext(nc) as tc:
        with (
            tc.tile_pool(name="consts", bufs=1) as consts,
            tc.tile_pool(name="wpool", bufs=3) as wpool,
            tc.tile_pool(name="qkpool", bufs=1) as qkpool,
            tc.tile_pool(name="vpool", bufs=1) as vpool,
            tc.tile_pool(name="shpool", bufs=2) as shpool,
            tc.tile_pool(name="ptpool", bufs=3) as ptpool,
            tc.tile_pool(name="ypool", bufs=1) as ypool,
            tc.tile_pool(name="stgpool", bufs=2) as stgpool,
            tc.tile_pool(name="rdbcpool", bufs=2) as rdbcpool,
            tc.tile_pool(name="psP", bufs=2, space="PSUM") as psP,
            tc.tile_pool(name="psS", bufs=2, space="PSUM") as psS,
            tc.tile_pool(name="dramp", bufs=1, space="DRAM") as dramp,
        ):
            # ---- constants ----
            # cos/sin duplicated along a co-pair axis so rotary runs on
            # [128, 2, TP] blocks (two c_out tiles at once).
            cos1 = consts.tile([128, TP], bf, tag="cos1")
            sin1 = consts.tile([128, TP], bf, tag="sin1")
            nc.scalar.dma_start(out=cos1, in_=cos_d[:, :])
            nc.scalar.dma_start(out=sin1, in_=sin_d[:, :])
            m0_sb = consts.tile([128, 128], bf, tag="m0")
            nc.scalar.dma_start(out=m0_sb, in_=m0_d[:, :])
            bq_sb = consts.tile([128, NCO], f32, tag="bq")
            bk_sb = consts.tile([128, NCO], f32, tag="bk")
            bp_sb = consts.tile([128, NCO], f32, tag="bp")
            nc.scalar.dma_start(out=bq_sb, in_=bq_d[:, :])
            nc.scalar.dma_start(out=bk_sb, in_=bk_d[:, :])
            nc.scalar.dma_start(out=bp_sb, in_=bp_d[:, :])
            ones16 = consts.tile([128, 16], bf, tag="ones16")
            nc.vector.memset(ones16, 1.0)
            zcol = consts.tile([128, 128], bf, tag="zcol")
            nc.vector.memset(zcol, 0.0)
            bv_sb = consts.tile([128, C], bf, tag="bv")

            dnd = dramp.tile([H, TP], f32, tag="dnd")

            # ---- persistent activations ----
            qT = qkpool.tile([128, NCI, TP], bf, tag="qT")
            kT = qkpool.tile([128, NCI, TP], bf, tag="kT")
            vaug = vpool.tile([128, NTT, VW], bf, tag="vaug")



            # ---- S^T + exp dose machinery ----
            # Each dose is ~0.2 us of PE work gated on one psS slot pair; the
            # pump() calls sprinkle them through the projection emission at
            # roughly the scalar engine's exp drain rate.
            st_queue = []
            pair_pts = [dict() for _ in range(NCI)]
            pair_done = [False] * NCI

            def pump(n=1):
                # Emit all available doses eagerly — the Tile scheduler
                # reorders by readiness, so a dose waiting on an exp slot
                # doesn't block projection matmuls behind it.
                while st_queue:
                    st_queue.pop(0)()

            def ensure_pair(j):
                while not pair_done[j]:
                    st_queue.pop(0)()

            def dose_nk(j, nk):
                # pt tiles hold local q coords [qlo:770) -> [0:770-qlo) so the
                # pool slots are exactly sized (deeper pipeline per KB).
                qlo, moff = R0SUB[nk]
                w = TP - qlo
                sta = psS.tile([128, 1024], f32, tag="st", name=f"sta{j}_{nk}")
                stb = psS.tile([128, 1024], f32, tag="st", name=f"stb{j}_{nk}")
                ks = slice(nk * 128, (nk + 1) * 128)
                nc.tensor.matmul(sta[:, qlo:512], kT[0:64, j, ks],
                                 qT[0:64, j, qlo:512], start=True, stop=True)
                nc.tensor.matmul(sta[:, 512:770], kT[0:64, j, ks],
                                 qT[0:64, j, 512:770], start=True, stop=True)
                nc.tensor.matmul(stb[:, qlo:512], kT[64:128, j, ks],
                                 qT[64:128, j, qlo:512], start=True, stop=True)
                nc.tensor.matmul(stb[:, 512:770], kT[64:128, j, ks],
                                 qT[64:128, j, 512:770], start=True, stop=True)
                pa = ptpool.tile([128, w], bf, tag=f"pta{nk}",
                                 name=f"pta{j}_{nk}")
                pb = ptpool.tile([128, w], bf, tag=f"ptb{nk}",
                                 name=f"ptb{j}_{nk}")
                nc.scalar.activation(out=pa[:, 0:w], in_=sta[:, qlo:770],
                                     func=Exp, scale=0.125)
                nc.scalar.activation(out=pb[:, 0:w], in_=stb[:, qlo:770],
                                     func=Exp, scale=0.125)
                if moff is not None:
                    nc.gpsimd.tensor_mul(pa[:, 0:128], pa[:, 0:128], m0_sb)
                    nc.gpsimd.tensor_mul(pb[:, 0:128], pb[:, 0:128], m0_sb)
                pair_pts[j]["a", nk] = pa
                pair_pts[j]["b", nk] = pb

            def dose_tail(j):
                # kv col 768 (single kv row); q col 512 is masked -> zeroed.
                st6a = psS.tile([128, 1024], f32, tag="st", name=f"st6a{j}")
                st6b = psS.tile([128, 1024], f32, tag="st", name=f"st6b{j}")
                nc.tensor.matmul(st6a[0:1, 0:258], kT[0:64, j, 768:769],
                                 qT[0:64, j, 512:770], start=True, stop=True)
                nc.tensor.matmul(st6b[0:1, 0:258], kT[64:128, j, 768:769],
                                 qT[64:128, j, 512:770], start=True, stop=True)
                p6a = ptpool.tile([1, 258], bf, tag="pt6a", name=f"pt6a{j}")
                p6b = ptpool.tile([1, 258], bf, tag="pt6b", name=f"pt6b{j}")
                nc.scalar.activation(out=p6a[0:1, 1:258],
                                     in_=st6a[0:1, 1:258], func=Exp,
                                     scale=0.125)
                nc.scalar.activation(out=p6b[0:1, 1:258],
                                     in_=st6b[0:1, 1:258], func=Exp,
                                     scale=0.125)
                nc.vector.tensor_copy(p6a[0:1, 0:1], zcol[0:1, 0:1])
                nc.vector.tensor_copy(p6b[0:1, 0:1], zcol[0:1, 0:1])
                pair_pts[j]["a", 6] = p6a
                pair_pts[j]["b", 6] = p6b
                pair_done[j] = True

            def push_pair(j):
                for nk in range(6):
                    st_queue.append(lambda j=j, nk=nk: dose_nk(j, nk))
                st_queue.append(lambda j=j: dose_tail(j))

            # ---- Q/K projection (in [c_out, t] layout) + rotary ----
            def emit_qk_proj(w, x, b_sb, outT_t, co, nm):
                ps = psP.tile([128, 1024], f32, tag="ps", name=f"{nm}P{co}")
                for ci in range(NCI):
                    lhs = w[:, co, ci, :]
                    nc.tensor.matmul(ps[:, 0:512], lhs, x[:, ci, 0:512],
                                     start=(ci == 0), stop=(ci == NCI - 1))
                    nc.tensor.matmul(ps[:, 512:770], lhs, x[:, ci, 512:770],
                                     start=(ci == 0), stop=(ci == NCI - 1))
                    if ci in (2, 5):
                        pump()
                nc.vector.tensor_scalar_add(
                    outT_t[:, co, :], ps[:, 0:770], b_sb[:, co:co + 1])

            def emit_rot(outT_t, co, nm):
                # partial rotary on the (host-permuted) first 32 dims of each
                # head: 16-row partition-block swap + q*cos + swapped*sin.
                # cos rows outside the rotary dims are 1.0, sin rows 0.0.
                # sh swaps ride the scalar queue, which drains its share of
                # the input burst early — they never sit behind multi-MB
                # weight transfers there.
                q = nc.sync if nm == "q" else nc.gpsimd
                blk = outT_t[:, co, :]
                sh = shpool.tile([128, TP], bf, tag="sh", name=f"sh{nm}{co}")
                q.dma_start(out=sh[32:64], in_=blk[32:64])
                for s in (0, 64):
                    q.dma_start(out=sh[s:s + 16], in_=blk[s + 16:s + 32])
                    q.dma_start(out=sh[s + 16:s + 32], in_=blk[s:s + 16])
                nc.vector.tensor_mul(sh[0:96], sh[0:96], sin1[0:96])
                nc.vector.tensor_mul(blk, blk, cos1)
                nc.vector.tensor_add(blk[0:96], blk[0:96], sh[0:96])

            # xq/xkv are dead after the V projection; scoping them lets the
            # yT / out-staging pools reuse the same SBUF region.
            xpool_cm = tc.tile_pool(name="xpool", bufs=1)
            xpool = xpool_cm.__enter__()
            xq = xpool.tile([128, NCI, TP], bf, tag="xq")
            xkv = xpool.tile([128, NCI, TP], bf, tag="xkv")
            # Startup burst spread across all five engine DMA queues (each is
            # ~85 GB/s): x and Q/K weights land by ~18 us so the exp chain can
            # start ~30 us in. Waiting triggers (wp reuses wq's pool slot)
            # stay off the tensor/vector/gpsimd queues — a waiting trigger
            # head-blocks everything behind it in that queue.
            # Only sync/scalar/gpsimd engines can trigger DMAs (~85 GB/s per
            # queue). Upfront: just the wq/wk co 0-1 slices, the x tensors
            # (1-ci chunks round-robined over all three queues) and consts —
            # everything else trickles in per-co inside the loop so the
            # latency-critical rotary sh swaps never queue behind multi-MB
            # transfers. The kernel is paced by the exp chain (~10 us/pair),
            # so later weights have ample queue time.
            wq = wpool.tile([128, NCO, NCI, 128], bf, tag="w", name="wq")
            wk = wpool.tile([128, NCO, NCI, 128], bf, tag="w", name="wk")
            wv = wpool.tile([128, NCI, C], bf, tag="w", name="wv")
            nc.sync.dma_start(out=wq[:, 0:2], in_=wq_d[:, 0:2])
            nc.gpsimd.dma_start(out=wk[:, 0:2], in_=wk_d[:, 0:2])
            # all xq chunks land first (Q co0 needs the full tensor), xkv
            # behind them — Q co0 starts ~13 us in, K co0 ~20 us.
            qs3 = (nc.sync, nc.gpsimd, nc.scalar)
            for ci in range(NCI):
                qs3[ci % 3].dma_start(out=xq[:, ci], in_=xqT_d[:, ci])
            for ci in range(NCI):
                qs3[ci % 3].dma_start(out=xkv[:, ci], in_=xkvT_d[:, ci])
            # ---- V projection (in [t, c_out] layout, ones-augmented) ----
            def emit_v_group(tg):
                tiles = {}
                for tt in tg:
                    tiles[tt] = psP.tile([128, 1024], f32, tag="ps",
                                         name=f"v{tt}")
                for ci in range(NCI):
                    for tt in tg:
                        tsz = 128 if tt < 6 else 1
                        lhs = xkv[:, ci, tt * 128:tt * 128 + tsz]
                        for hf in (0, 1):
                            nc.tensor.matmul(
                                tiles[tt][:tsz, hf * 512:(hf + 1) * 512], lhs,
                                wv[:, ci, hf * 512:(hf + 1) * 512],
                                start=(ci == 0), stop=(ci == NCI - 1))
                for tt in tg:
                    tsz = 128 if tt < 6 else 1
                    va = vaug[:tsz, tt, 0:H * (HD + 1)].rearrange(
                        "p (h e) -> p h e", e=HD + 1)
                    nc.vector.tensor_add(
                        va[:, :, 0:HD],
                        tiles[tt][:tsz, :].rearrange("p (h d) -> p h d", h=H),
                        bv_sb[:tsz, :].rearrange("p (h d) -> p h d", h=H))
                    nc.vector.tensor_copy(
                        va[:, :, HD:HD + 1], ones16[:tsz, :].unsqueeze(2))
                    # pad tail so M=128 lhsT slices stay initialized
                    nc.vector.tensor_copy(
                        vaug[:tsz, tt, H * (HD + 1):VW],
                        zcol[:tsz, 0:VW - H * (HD + 1)])

            yT = ypool.tile([128, NCI, TP], bf, tag="yT")

            # ---- attention PV + softmax divide ----
            def emit_pv(j, side):
                h = 2 * j + (0 if side == "a" else 1)
                vs = slice(h * (HD + 1), h * (HD + 1) + 128)
                pts = pair_pts[j]
                o = psP.tile([128, 1024], f32, tag="ps", name=f"pv{h}")
                for nk in range(6):
                    qlo, _ = R0SUB[nk]
                    p = pts[side, nk]
                    nc.tensor.matmul(o[:, qlo:512], vaug[:, nk, vs],
                                     p[:, 0:512 - qlo], start=(nk == 0),
                                     stop=False)
                    nc.tensor.matmul(o[:, 512:770], vaug[:, nk, vs],
                                     p[:, 512 - qlo:TP - qlo],
                                     start=(nk == 0), stop=False)
                p6 = pts[side, 6]
                nc.tensor.matmul(o[:, 512:770], vaug[0:1, 6, vs],
                                 p6[0:1, 0:258], start=False, stop=True)
                return o

            def emit_div(j, side, o, stg):
                # release the PSUM slot quickly: a plain copy into yT plus an
                # ACT/DVE-copy of the denominator row to SBUF staging (DMA
                # cannot read PSUM); the divide happens in-place on yT once
                # the broadcast lands.
                h = 2 * j + (0 if side == "a" else 1)
                r = slice(0, 64) if side == "a" else slice(64, 128)
                srow = 0 if side == "a" else 1
                nc.vector.tensor_copy(yT[r, j, :], o[0:64, 0:770])
                if side == "a":  # split staging between ACT and DVE so the
                    # exp chain on ACT only absorbs half of it
                    nc.scalar.activation(out=stg[0:1, srow, :],
                                         in_=o[HD:HD + 1, 0:770], func=Copy)
                else:
                    nc.vector.tensor_copy(stg[0:1, srow, :],
                                          o[HD:HD + 1, 0:770])
                nc.gpsimd.dma_start(out=dnd[h:h + 1, :], in_=stg[0:1, srow, :])

            def emit_pv_pair(j):
                ensure_pair(j)
                stg = stgpool.tile([1, 2, TP], f32, tag="stg", name=f"stg{j}")
                oa = emit_pv(j, "a")
                emit_div(j, "a", oa, stg)
                ob = emit_pv(j, "b")
                emit_div(j, "b", ob, stg)
                pump()
                # denominator rows broadcast across partitions via DRAM, one
                # reciprocal for the pair, then the in-place divide of yT.
                rdbc = rdbcpool.tile([128, TP], f32, tag="rdbc",
                                     name=f"rdbc{j}")
                nc.gpsimd.dma_start(
                    out=rdbc[0:64, :],
                    in_=dnd[2 * j:2 * j + 1, :].broadcast_to((64, TP)))
                nc.gpsimd.dma_start(
                    out=rdbc[64:128, :],
                    in_=dnd[2 * j + 1:2 * j + 2, :].broadcast_to((64, TP)))
                nc.vector.reciprocal_approx_fast(out=rdbc, in_=rdbc)
                nc.vector.tensor_mul(yT[0:64, j, :], yT[0:64, j, :],
                                     rdbc[0:64, :])
                nc.vector.tensor_mul(yT[64:128, j, :], yT[64:128, j, :],
                                     rdbc[64:128, :])

            # V tt-groups go at co 2-5 (wv trickles in during co 0-1) so vaug
            # completes mid-loop, and PV pairs 0-3 interleave at co 4-7 —
            # freeing pt slots so pairs 4-7's exps never wait on PV.
            v_at = {2: [(0, 1)], 3: [(2, 3)], 4: [(4, 5), (6,)]}
            pv_at = {4: 0, 5: 1, 6: 2, 7: 3}
            for co in range(NCO):
                emit_qk_proj(wq, xq, bq_sb, qT, co, "q")
                emit_rot(qT, co, "q")
                if co < NCO - 2:  # trickle the remaining weight columns
                    nc.sync.dma_start(out=wq[:, co + 2], in_=wq_d[:, co + 2])
                pump()
                emit_qk_proj(wk, xkv, bk_sb, kT, co, "k")
                emit_rot(kT, co, "k")
                if co < NCO - 2:
                    nc.gpsimd.dma_start(out=wk[:, co + 2], in_=wk_d[:, co + 2])
                if co < 2:  # wv fully needed by the first V group (co 2)
                    for i in range(4):
                        q = nc.sync if i % 2 == 0 else nc.gpsimd
                        q.dma_start(out=wv[:, 4 * co + i],
                                    in_=wv_d[:, 4 * co + i])
                if co == 1:
                    nc.gpsimd.dma_start(
                        out=bv_sb, in_=bv_d[0:1, :].broadcast_to((128, C)))
                push_pair(co)
                pump()
                for g in v_at.get(co, ()):
                    emit_v_group(g)
                if co in pv_at:
                    emit_pv_pair(pv_at[co])
            # wp reuses wq's slot; its triggers wait for the Q projection's
            # last weight read, so they are emitted after everything else
            # that shares the sync queue in this phase.
            wp = wpool.tile([128, NCI, C], bf, tag="w", name="wp")
            nc.sync.dma_start(out=wp[:, 0:4], in_=wp_d[:, 0:4])
            nc.sync.dma_start(out=wp[:, 4:NCI], in_=wp_d[:, 4:NCI])
            xpool_cm.__exit__(None, None, None)
            otpool_cm = tc.tile_pool(name="otpool", bufs=2)
            otpool = otpool_cm.__enter__()
            yO = otpool.tile([128, NCO, TP], bf, tag="yO", bufs=1)

            # ---- output projection (in [c_out, t] layout), split in two ----
            # Pass 1 (ci 0-5) runs as soon as pair 5's divide lands — it
            # overlaps the attention tail and keeps the PE warm. Pass 2 adds
            # the last two ci after the final divide, so the post-attention
            # serial tail is just 32 matmuls + the adds. bf16 output (+0.4%
            # rounding) and stores rotate over all three DMA queues.
            emit_pv_pair(4)
            emit_pv_pair(5)
            for co in range(NCO):
                ps = psP.tile([128, 1024], f32, tag="ps", name=f"oP{co}")
                for ci in range(6):
                    lhs = wp[:, ci, co * 128:(co + 1) * 128]
                    nc.tensor.matmul(ps[:, 0:512], lhs, yT[:, ci, 0:512],
                                     start=(ci == 0), stop=(ci == 5))
                    nc.tensor.matmul(ps[:, 512:770], lhs, yT[:, ci, 512:770],
                                     start=(ci == 0), stop=(ci == 5))
                nc.vector.tensor_scalar_add(
                    yO[:, co, :], ps[:, 0:770], bp_sb[:, co:co + 1])
            emit_pv_pair(6)
            emit_pv_pair(7)
            for co in range(NCO):
                ps = psP.tile([128, 1024], f32, tag="ps", name=f"oQ{co}")
                for ci in (6, 7):
                    lhs = wp[:, ci, co * 128:(co + 1) * 128]
                    nc.tensor.matmul(ps[:, 0:512], lhs, yT[:, ci, 0:512],
                                     start=(ci == 6), stop=(ci == 7))
                    nc.tensor.matmul(ps[:, 512:770], lhs, yT[:, ci, 512:770],
                                     start=(ci == 6), stop=(ci == 7))
                ot = otpool.tile([128, TP], bf, tag="ot", name=f"ot{co}")
                nc.vector.tensor_add(ot[:, :], ps[:, 0:770], yO[:, co, :])
                qs3[co % 3].dma_start(
                    out=outT_d[co * 128:(co + 1) * 128, :], in_=ot[:, :])

            otpool_cm.__exit__(None, None, None)

    nc.compile()
    return nc


def _host_prep(x_q, x_kv, rotary_pos_emb, Wq, bq, Wk, bk, Wv, bv, Wp, bp):
    import ml_dtypes

    f = np.float32
    bfl = ml_dtypes.bfloat16
    x_q = np.asarray(x_q, f)
    x_kv = np.asarray(x_kv, f)
    freqs = np.asarray(rotary_pos_emb, f)

    # Even/odd pair-split permutation of the first 32 dims of each head, so
    # rotate_half becomes a 16-partition block swap on chip.
    perm = np.arange(C)
    for h in range(H):
        b0 = h * HD
        blk = np.empty(HD, np.int64)
        blk[0:16] = b0 + np.arange(0, 32, 2)
        blk[16:32] = b0 + np.arange(1, 32, 2)
        blk[32:64] = b0 + np.arange(32, 64)
        perm[b0:b0 + HD] = blk

    def wT(W, p=None):
        W = np.asarray(W, f)
        if p is not None:
            W = W[p, :]
        return np.ascontiguousarray(W.T).astype(bfl)

    cosE = np.cos(freqs[:, 0::2]).T  # [16, T]
    cosO = np.cos(freqs[:, 1::2]).T
    sinE = -np.sin(freqs[:, 0::2]).T
    sinO = np.sin(freqs[:, 1::2]).T
    cosP = np.ones((128, TP), f)
    sinP = np.zeros((128, TP), f)
    for s in (0, 64):
        cosP[s:s + 16, :T] = cosE
        cosP[s + 16:s + 32, :T] = cosO
        sinP[s:s + 16, :T] = sinE
        sinP[s + 16:s + 32, :T] = sinO


    p_idx = np.arange(128)[:, None]
    f_idx = np.arange(128)[None, :]
    m0 = (p_idx < f_idx).astype(f)

    def tile_qk(wt):  # [C, C] (ci, co) -> [p, co, ci, 128]
        return np.ascontiguousarray(
            wt.reshape(NCI, 128, NCO, 128).transpose(1, 2, 0, 3))

    def tile_vp(wt):  # [C, C] (ci, co) -> [p, ci, C]
        return np.ascontiguousarray(wt.reshape(NCI, 128, C).transpose(1, 0, 2))

    bqp = np.asarray(bq, f)[perm]
    bkp = np.asarray(bk, f)[perm]
    shared = {
        "wqT3": tile_qk(wT(Wq, perm)),
        "wkT3": tile_qk(wT(Wk, perm)),
        "wvT3": tile_vp(wT(Wv)),
        "wpT3": tile_vp(wT(Wp)),
        "bq2": np.ascontiguousarray(bqp.reshape(NCO, 128).T),
        "bk2": np.ascontiguousarray(bkp.reshape(NCO, 128).T),
        "bp2": np.ascontiguousarray(np.asarray(bp, f).reshape(NCO, 128).T),
        "bv1": np.asarray(bv, f).reshape(1, C).astype(bfl),
        "cosP": np.ascontiguousarray(cosP).astype(bfl),
        "sinP": np.ascontiguousarray(sinP).astype(bfl),
        "m0": np.ascontiguousarray(m0).astype(bfl),
    }

    def padT(xt):  # [C, T] -> [p, ci, TP]
        out = np.zeros((C, TP), f)
        out[:, :T] = xt
        return np.ascontiguousarray(
            out.reshape(NCI, 128, TP).transpose(1, 0, 2)).astype(bfl)

    in_maps = []
    for b in range(B):
        m = dict(shared)
        m["xqT3"] = padT(x_q[b].T)
        m["xkvT3"] = padT(x_kv[b].T)
        in_maps.append(m)
    return in_maps


def kernel(x_q, x_kv, rotary_pos_emb, Wq, bq, Wk, bk, Wv, bv, Wp, bp):
    from concourse.bass_utils import run_bass_kernel_spmd

    if "nc" not in _CACHE:
        _CACHE["nc"] = _build_program()
    nc = _CACHE["nc"]

    in_maps = _host_prep(x_q, x_kv, rotary_pos_emb,
                         Wq, bq, Wk, bk, Wv, bv, Wp, bp)
    trace = os.environ.get("BTK_TRACE", "0") == "1"
    res = run_bass_kernel_spmd(
        nc, in_maps, core_ids=list(range(B)), trace=trace)
    _CACHE["last_result"] = res
    return np.stack(
        [np.ascontiguousarray(r["outT"][:, :T].T).astype(np.float32)
         for r in res.results],
        axis=0)


# revision 94
# speedup vs baseline: 1.2347x; 1.2347x over previous
"""Trainium2 Bass kernel for CausalCrossAttention (B=8, T=769, C=1024, H=16).

Sharding: data-parallel over batch B=8 across the 8 NeuronCores (one batch
element per core, SPMD — identical program, different input slices).

v2 — all-bf16 matmul operands (fp32 PSUM accumulation), designed around the
trace of the fp32r v1 (340 us):
  * bf16 stationary operands get Fast Weight Load (fp32 LDWEIGHTS was ~45% of
    PE busy time in v1) and bf16 moving operands avoid the fp32r 4-cycle/row
    penalty on free dims < 256 (the attention boundary tiles).
  * S^T per head-pair via K=64 row-tiled matmul pairs (tile_position rows 0/64
    run concurrently in the PE array) -> one N-stream per pair instead of one
    per head, and no sibling-row zeroing.
  * The softmax exp chain on the scalar engine (~76 us serial) is the #2
    resource after the PE, so S^T+exp "doses" are interleaved into the Q/K/V
    projection emission to start it ~7 us in instead of after projections.
  * Output projection in [c_out, t] layout (host un-transposes) -> same cost
    as Q/K and no M=1 tail streams.
  * PV PSUM slots are released by plain copies into yT right after the PV
    matmuls; the softmax divide (denominator row -> DRAM -> partition
    broadcast -> reciprocal -> in-place mul) runs off the critical path.

PSUM budget: psP 4x[128,512] (projections, PV, out-proj) + psS 2x[128,1024]
(S^T tiles, exp-consumed) = 8 banks.
"""

import os

import numpy as np

B, T, C = 8, 769, 1024
H, HD, L = 16, 64, 32
COND = 256
NCI = 8  # 1024 / 128 contraction tiles
NCO = 8
NTT = 7  # t tiles: 6 full + 1 single row
TP = 770  # streamed T padded to even
VW = H * (HD + 1) + 63  # V_aug free width, padded so M=128 slices stay in-bounds

# Per-(kv-tile) q ranges in the 0:512 block + mask offset.
# nk covers kv cols [128*nk, 128*nk+128); allowed iff kv_col < 256 + q_col,
# i.e. p < f + 256 - 128*nk with p the in-tile kv index, f the abs q col.
R0SUB = {0: (0, None), 1: (0, None), 2: (0, 0),
         3: (128, 128), 4: (256, 256), 5: (384, 384)}

_CACHE = {}


def _build_program():
    import concourse.mybir as mybir
    import concourse.tile as tile
    from concourse import bacc

    f32 = mybir.dt.float32
    bf = mybir.dt.bfloat16
    Exp = mybir.ActivationFunctionType.Exp
    Ident = mybir.ActivationFunctionType.Identity
    Copy = mybir.ActivationFunctionType.Copy

    nc = bacc.Bacc("TRN2", target_bir_lowering=False)

    # All big inputs are host-pre-tiled to the SBUF layout [partition, free...]
    # so each DMA moves one large contiguous chunk per partition (the DMA
    # queues are descriptor-rate bound, ~1 descriptor per (partition, chunk)).
    xqT_d = nc.dram_tensor("xqT3", [128, NCI, TP], bf, kind="ExternalInput")
    xkvT_d = nc.dram_tensor("xkvT3", [128, NCI, TP], bf, kind="ExternalInput")
    # Q/K weights co-major [p, co, ci, 128] so the first co pair's slice is
    # one small leading DMA; V/P weights ci-major [p, ci, co].
    wq_d = nc.dram_tensor("wqT3", [128, NCO, NCI, 128], bf,
                          kind="ExternalInput")
    wk_d = nc.dram_tensor("wkT3", [128, NCO, NCI, 128], bf,
                          kind="ExternalInput")
    wv_d = nc.dram_tensor("wvT3", [128, NCI, C], bf, kind="ExternalInput")
    wp_d = nc.dram_tensor("wpT3", [128, NCI, C], bf, kind="ExternalInput")
    bq_d = nc.dram_tensor("bq2", [128, NCO], f32, kind="ExternalInput")
    bk_d = nc.dram_tensor("bk2", [128, NCO], f32, kind="ExternalInput")
    bp_d = nc.dram_tensor("bp2", [128, NCO], f32, kind="ExternalInput")
    bv_d = nc.dram_tensor("bv1", [1, C], bf, kind="ExternalInput")
    cos_d = nc.dram_tensor("cosP", [128, TP], bf, kind="ExternalInput")
    sin_d = nc.dram_tensor("sinP", [128, TP], bf, kind="ExternalInput")
    m0_d = nc.dram_tensor("m0", [128, 128], bf, kind="ExternalInput")
    outT_d = nc.dram_tensor("outT", [C, TP], bf, kind="ExternalOutput")

    with tile.TileContext(nc) as tc:
        with (
            tc.tile_pool(name="consts", bufs=1) as consts,
            tc.tile_pool(name="wpool", bufs=3) as wpool,
            tc.tile_pool(name="qkpool", bufs=1) as qkpool,
            tc.tile_pool(name="vpool", bufs=1) as vpool,
            tc.tile_pool(name="shpool", bufs=2) as shpool,
            tc.tile_pool(name="ptpool", bufs=3) as ptpool,
            tc.tile_pool(name="ypool", bufs=1) as ypool,
            tc.tile_pool(name="stgpool", bufs=2) as stgpool,
            tc.tile_pool(name="rdbcpool", bufs=2) as rdbcpool,
            tc.tile_pool(name="psP", bufs=2, space="PSUM") as psP,
            tc.tile_pool(name="psS", bufs=2, space="PSUM") as psS,
            tc.tile_pool(name="dramp", bufs=1, space="DRAM") as dramp,
        ):
            # ---- constants ----
            # cos/sin duplicated along a co-pair axis so rotary runs on
            # [128, 2, TP] blocks (two c_out tiles at once).
            cos1 = consts.tile([128, TP], bf, tag="cos1")
            sin1 = consts.tile([128, TP], bf, tag="sin1")
            nc.scalar.dma_start(out=cos1, in_=cos_d[:, :])
            nc.scalar.dma_start(out=sin1, in_=sin_d[:, :])
            m0_sb = consts.tile([128, 128], bf, tag="m0")
            nc.scalar.dma_start(out=m0_sb, in_=m0_d[:, :])
            bq_sb = consts.tile([128, NCO], f32, tag="bq")
            bk_sb = consts.tile([128, NCO], f32, tag="bk")
            bp_sb = consts.tile([128, NCO], f32, tag="bp")
            nc.scalar.dma_start(out=bq_sb, in_=bq_d[:, :])
            nc.scalar.dma_start(out=bk_sb, in_=bk_d[:, :])
            nc.scalar.dma_start(out=bp_sb, in_=bp_d[:, :])
            ones16 = consts.tile([128, 16], bf, tag="ones16")
            nc.vector.memset(ones16, 1.0)
            zcol = consts.tile([128, 128], bf, tag="zcol")
            nc.vector.memset(zcol, 0.0)
            bv_sb = consts.tile([128, C], bf, tag="bv")

            dnd = dramp.tile([H, TP], f32, tag="dnd")

            # ---- persistent activations ----
            qT = qkpool.tile([128, NCI, TP], bf, tag="qT")
            kT = qkpool.tile([128, NCI, TP], bf, tag="kT")
            vaug = vpool.tile([128, NTT, VW], bf, tag="vaug")



            # ---- S^T + exp dose machinery ----
            # Each dose is ~0.2 us of PE work gated on one psS slot pair; the
            # pump() calls sprinkle them through the projection emission at
            # roughly the scalar engine's exp drain rate.
            st_queue = []
            pair_pts = [dict() for _ in range(NCI)]
            pair_done = [False] * NCI

            def pump(n=1):
                # Emit all available doses eagerly — the Tile scheduler
                # reorders by readiness, so a dose waiting on an exp slot
                # doesn't block projection matmuls behind it.
                while st_queue:
                    st_queue.pop(0)()

            def ensure_pair(j):
                while not pair_done[j]:
                    st_queue.pop(0)()

            def dose_nk(j, nk):
                # pt tiles hold local q coords [qlo:770) -> [0:770-qlo) so the
                # pool slots are exactly sized (deeper pipeline per KB).
                qlo, moff = R0SUB[nk]
                w = TP - qlo
                sta = psS.tile([128, 1024], f32, tag="st", name=f"sta{j}_{nk}")
                stb = psS.tile([128, 1024], f32, tag="st", name=f"stb{j}_{nk}")
                ks = slice(nk * 128, (nk + 1) * 128)
                nc.tensor.matmul(sta[:, qlo:512], kT[0:64, j, ks],
                                 qT[0:64, j, qlo:512], start=True, stop=True)
                nc.tensor.matmul(sta[:, 512:770], kT[0:64, j, ks],
                                 qT[0:64, j, 512:770], start=True, stop=True)
                nc.tensor.matmul(stb[:, qlo:512], kT[64:128, j, ks],
                                 qT[64:128, j, qlo:512], start=True, stop=True)
                nc.tensor.matmul(stb[:, 512:770], kT[64:128, j, ks],
                                 qT[64:128, j, 512:770], start=True, stop=True)
                pa = ptpool.tile([128, w], bf, tag=f"pta{nk}",
                                 name=f"pta{j}_{nk}")
                pb = ptpool.tile([128, w], bf, tag=f"ptb{nk}",
                                 name=f"ptb{j}_{nk}")
                nc.scalar.activation(out=pa[:, 0:w], in_=sta[:, qlo:770],
                                     func=Exp, scale=0.125)
                nc.scalar.activation(out=pb[:, 0:w], in_=stb[:, qlo:770],
                                     func=Exp, scale=0.125)
                if moff is not None:
                    nc.gpsimd.tensor_mul(pa[:, 0:128], pa[:, 0:128], m0_sb)
                    nc.gpsimd.tensor_mul(pb[:, 0:128], pb[:, 0:128], m0_sb)
                pair_pts[j]["a", nk] = pa
                pair_pts[j]["b", nk] = pb

            def dose_tail(j):
                # kv col 768 (single kv row); q col 512 is masked -> zeroed.
                st6a = psS.tile([128, 1024], f32, tag="st", name=f"st6a{j}")
                st6b = psS.tile([128, 1024], f32, tag="st", name=f"st6b{j}")
                nc.tensor.matmul(st6a[0:1, 0:258], kT[0:64, j, 768:769],
                                 qT[0:64, j, 512:770], start=True, stop=True)
                nc.tensor.matmul(st6b[0:1, 0:258], kT[64:128, j, 768:769],
                                 qT[64:128, j, 512:770], start=True, stop=True)
                p6a = ptpool.tile([1, 258], bf, tag="pt6a", name=f"pt6a{j}")
                p6b = ptpool.tile([1, 258], bf, tag="pt6b", name=f"pt6b{j}")
                nc.scalar.activation(out=p6a[0:1, 1:258],
                                     in_=st6a[0:1, 1:258], func=Exp,
                                     scale=0.125)
                nc.scalar.activation(out=p6b[0:1, 1:258],
                                     in_=st6b[0:1, 1:258], func=Exp,
                                     scale=0.125)
                nc.vector.tensor_copy(p6a[0:1, 0:1], zcol[0:1, 0:1])
                nc.vector.tensor_copy(p6b[0:1, 0:1], zcol[0:1, 0:1])
                pair_pts[j]["a", 6] = p6a
                pair_pts[j]["b", 6] = p6b
                pair_done[j] = True

            def push_pair(j):
                for nk in range(6):
                    st_queue.append(lambda j=j, nk=nk: dose_nk(j, nk))
                st_queue.append(lambda j=j: dose_tail(j))

            # ---- Q/K projection (in [c_out, t] layout) + rotary ----
            def emit_qk_proj(w, x, b_sb, outT_t, co, nm):
                ps = psP.tile([128, 1024], f32, tag="ps", name=f"{nm}P{co}")
                for ci in range(NCI):
                    lhs = w[:, co, ci, :]
                    nc.tensor.matmul(ps[:, 0:512], lhs, x[:, ci, 0:512],
                                     start=(ci == 0), stop=(ci == NCI - 1))
                    nc.tensor.matmul(ps[:, 512:770], lhs, x[:, ci, 512:770],
                                     start=(ci == 0), stop=(ci == NCI - 1))
                    if ci in (2, 5):
                        pump()
                nc.vector.tensor_scalar_add(
                    outT_t[:, co, :], ps[:, 0:770], b_sb[:, co:co + 1])

            def emit_rot(outT_t, co, nm):
                # partial rotary on the (host-permuted) first 32 dims of each
                # head: 16-row partition-block swap + q*cos + swapped*sin.
                # cos rows outside the rotary dims are 1.0, sin rows 0.0.
                # sh swaps ride the scalar queue, which drains its share of
                # the input burst early — they never sit behind multi-MB
                # weight transfers there.
                q = nc.sync if nm == "q" else nc.gpsimd
                blk = outT_t[:, co, :]
                sh = shpool.tile([128, TP], bf, tag="sh", name=f"sh{nm}{co}")
                q.dma_start(out=sh[32:64], in_=blk[32:64])
                for s in (0, 64):
                    q.dma_start(out=sh[s:s + 16], in_=blk[s + 16:s + 32])
                    q.dma_start(out=sh[s + 16:s + 32], in_=blk[s:s + 16])
                nc.vector.tensor_mul(sh[0:96], sh[0:96], sin1[0:96])
                nc.vector.tensor_mul(blk, blk, cos1)
                nc.vector.tensor_add(blk[0:96], blk[0:96], sh[0:96])

            # xq/xkv are dead after the V projection; scoping them lets the
            # yT / out-staging pools reuse the same SBUF region.
            xpool_cm = tc.tile_pool(name="xpool", bufs=1)
            xpool = xpool_cm.__enter__()
            xq = xpool.tile([128, NCI, TP], bf, tag="xq")
            xkv = xpool.tile([128, NCI, TP], bf, tag="xkv")
            # Startup burst spread across all five engine DMA queues (each is
            # ~85 GB/s): x and Q/K weights land by ~18 us so the exp chain can
            # start ~30 us in. Waiting triggers (wp reuses wq's pool slot)
            # stay off the tensor/vector/gpsimd queues — a waiting trigger
            # head-blocks everything behind it in that queue.
            # Only sync/scalar/gpsimd engines can trigger DMAs (~85 GB/s per
            # queue). Upfront: just the wq/wk co 0-1 slices, the x tensors
            # (1-ci chunks round-robined over all three queues) and consts —
            # everything else trickles in per-co inside the loop so the
            # latency-critical rotary sh swaps never queue behind multi-MB
            # transfers. The kernel is paced by the exp chain (~10 us/pair),
            # so later weights have ample queue time.
            wq = wpool.tile([128, NCO, NCI, 128], bf, tag="w", name="wq")
            wk = wpool.tile([128, NCO, NCI, 128], bf, tag="w", name="wk")
            wv = wpool.tile([128, NCI, C], bf, tag="w", name="wv")
            nc.sync.dma_start(out=wq[:, 0:2], in_=wq_d[:, 0:2])
            nc.gpsimd.dma_start(out=wk[:, 0:2], in_=wk_d[:, 0:2])
            # all xq chunks land first (Q co0 needs the full tensor), xkv
            # behind them — Q co0 starts ~13 us in, K co0 ~20 us.
            qs3 = (nc.sync, nc.gpsimd, nc.scalar)
            for ci in range(NCI):
                qs3[ci % 3].dma_start(out=xq[:, ci], in_=xqT_d[:, ci])
            for ci in range(NCI):
                qs3[ci % 3].dma_start(out=xkv[:, ci], in_=xkvT_d[:, ci])
            # ---- V projection (in [t, c_out] layout, ones-augmented) ----
            def emit_v_group(tg):
                tiles = {}
                for tt in tg:
                    tiles[tt] = psP.tile([128, 1024], f32, tag="ps",
                                         name=f"v{tt}")
                for ci in range(NCI):
                    for tt in tg:
                        tsz = 128 if tt < 6 else 1
                        lhs = xkv[:, ci, tt * 128:tt * 128 + tsz]
                        for hf in (0, 1):
                            nc.tensor.matmul(
                                tiles[tt][:tsz, hf * 512:(hf + 1) * 512], lhs,
                                wv[:, ci, hf * 512:(hf + 1) * 512],
                                start=(ci == 0), stop=(ci == NCI - 1))
                for tt in tg:
                    tsz = 128 if tt < 6 else 1
                    va = vaug[:tsz, tt, 0:H * (HD + 1)].rearrange(
                        "p (h e) -> p h e", e=HD + 1)
                    nc.vector.tensor_add(
                        va[:, :, 0:HD],
                        tiles[tt][:tsz, :].rearrange("p (h d) -> p h d", h=H),
                        bv_sb[:tsz, :].rearrange("p (h d) -> p h d", h=H))
                    nc.vector.tensor_copy(
                        va[:, :, HD:HD + 1], ones16[:tsz, :].unsqueeze(2))
                    # pad tail so M=128 lhsT slices stay initialized
                    nc.vector.tensor_copy(
                        vaug[:tsz, tt, H * (HD + 1):VW],
                        zcol[:tsz, 0:VW - H * (HD + 1)])

            yT = ypool.tile([128, NCI, TP], bf, tag="yT")

            # ---- attention PV + softmax divide ----
            def emit_pv(j, side):
                h = 2 * j + (0 if side == "a" else 1)
                vs = slice(h * (HD + 1), h * (HD + 1) + 128)
                pts = pair_pts[j]
                o = psP.tile([128, 1024], f32, tag="ps", name=f"pv{h}")
                for nk in range(6):
                    qlo, _ = R0SUB[nk]
                    p = pts[side, nk]
                    nc.tensor.matmul(o[:, qlo:512], vaug[:, nk, vs],
                                     p[:, 0:512 - qlo], start=(nk == 0),
                                     stop=False)
                    nc.tensor.matmul(o[:, 512:770], vaug[:, nk, vs],
                                     p[:, 512 - qlo:TP - qlo],
                                     start=(nk == 0), stop=False)
                p6 = pts[side, 6]
                nc.tensor.matmul(o[:, 512:770], vaug[0:1, 6, vs],
                                 p6[0:1, 0:258], start=False, stop=True)
                return o

            def emit_div(j, side, o, stg):
                # release the PSUM slot quickly: a plain copy into yT plus an
                # ACT/DVE-copy of the denominator row to SBUF staging (DMA
                # cannot read PSUM); the divide happens in-place on yT once
                # the broadcast lands.
                h = 2 * j + (0 if side == "a" else 1)
                r = slice(0, 64) if side == "a" else slice(64, 128)
                srow = 0 if side == "a" else 1
                nc.vector.tensor_copy(yT[r, j, :], o[0:64, 0:770])
                if side == "a":  # split staging between ACT and DVE so the
                    # exp chain on ACT only absorbs half of it
                    nc.scalar.activation(out=stg[0:1, srow, :],
                                         in_=o[HD:HD + 1, 0:770], func=Copy)
                else:
                    nc.vector.tensor_copy(stg[0:1, srow, :],
                                          o[HD:HD + 1, 0:770])
                nc.gpsimd.dma_start(out=dnd[h:h + 1, :], in_=stg[0:1, srow, :])

            def emit_pv_pair(j):
                ensure_pair(j)
                stg = stgpool.tile([1, 2, TP], f32, tag="stg", name=f"stg{j}")
                oa = emit_pv(j, "a")
                emit_div(j, "a", oa, stg)
                ob = emit_pv(j, "b")
                emit_div(j, "b", ob, stg)
                pump()
                # denominator rows broadcast across partitions via DRAM, one
                # reciprocal for the pair, then the in-place divide of yT.
                rdbc = rdbcpool.tile([128, TP], f32, tag="rdbc",
                                     name=f"rdbc{j}")
                nc.gpsimd.dma_start(
                    out=rdbc[0:64, :],
                    in_=dnd[2 * j:2 * j + 1, :].broadcast_to((64, TP)))
                nc.gpsimd.dma_start(
                    out=rdbc[64:128, :],
                    in_=dnd[2 * j + 1:2 * j + 2, :].broadcast_to((64, TP)))
                nc.vector.reciprocal_approx_fast(out=rdbc, in_=rdbc)
                nc.vector.tensor_mul(yT[0:64, j, :], yT[0:64, j, :],
                                     rdbc[0:64, :])
                nc.vector.tensor_mul(yT[64:128, j, :], yT[64:128, j, :],
                                     rdbc[64:128, :])

            # V tt-groups go at co 2-5 (wv trickles in during co 0-1) so vaug
            # completes mid-loop, and PV pairs 0-3 interleave at co 4-7 —
            # freeing pt slots so pairs 4-7's exps never wait on PV.
            v_at = {2: [(0, 1)], 3: [(2, 3)], 4: [(4, 5), (6,)]}
            pv_at = {4: 0, 5: 1, 6: 2, 7: 3}
            for co in range(NCO):
                emit_qk_proj(wq, xq, bq_sb, qT, co, "q")
                emit_rot(qT, co, "q")
                if co < NCO - 2:  # trickle the remaining weight columns
                    nc.sync.dma_start(out=wq[:, co + 2], in_=wq_d[:, co + 2])
                pump()
                emit_qk_proj(wk, xkv, bk_sb, kT, co, "k")
                emit_rot(kT, co, "k")
                if co < NCO - 2:
                    nc.gpsimd.dma_start(out=wk[:, co + 2], in_=wk_d[:, co + 2])
                if co < 2:  # wv fully needed by the first V group (co 2)
                    for i in range(4):
                        q = nc.sync if i % 2 == 0 else nc.gpsimd
                        q.dma_start(out=wv[:, 4 * co + i],
                                    in_=wv_d[:, 4 * co + i])
                if co == 1:
                    nc.gpsimd.dma_start(
                        out=bv_sb, in_=bv_d[0:1, :].broadcast_to((128, C)))
                push_pair(co)
                pump()
                for g in v_at.get(co, ()):
                    emit_v_group(g)
                if co in pv_at:
                    emit_pv_pair(pv_at[co])
            # wp reuses wq's slot; its triggers wait for the Q projection's
            # last weight read, so they are emitted after everything else
            # that shares the sync queue in this phase.
            wp = wpool.tile([128, NCI, C], bf, tag="w", name="wp")
            nc.sync.dma_start(out=wp[:, 0:4], in_=wp_d[:, 0:4])
            nc.sync.dma_start(out=wp[:, 4:NCI], in_=wp_d[:, 4:NCI])
            xpool_cm.__exit__(None, None, None)
            otpool_cm = tc.tile_pool(name="otpool", bufs=2)
            otpool = otpool_cm.__enter__()

            for j in range(4, NCI):
                emit_pv_pair(j)

            # ---- output projection (in [c_out, t] layout) ----
            # bf16 output (+0.4% rounding, well within tolerance) and the
            # store DMAs rotate over all three queues so the endgame isn't
            # bound by one ~85 GB/s queue.
            for co in range(NCO):
                ps = psP.tile([128, 1024], f32, tag="ps", name=f"oP{co}")
                for ci in range(NCI):
                    lhs = wp[:, ci, co * 128:(co + 1) * 128]
                    nc.tensor.matmul(ps[:, 0:512], lhs, yT[:, ci, 0:512],
                                     start=(ci == 0), stop=(ci == NCI - 1))
                    nc.tensor.matmul(ps[:, 512:770], lhs, yT[:, ci, 512:770],
                                     start=(ci == 0), stop=(ci == NCI - 1))
                ot = otpool.tile([128, TP], bf, tag="ot", name=f"ot{co}")
                nc.scalar.activation(out=ot[:, :], in_=ps[:, 0:770],
                                     func=Ident, bias=bp_sb[:, co:co + 1],
                                     scale=1.0)
                qs3[co % 3].dma_start(
                    out=outT_d[co * 128:(co + 1) * 128, :], in_=ot[:, :])

            otpool_cm.__exit__(None, None, None)

    nc.compile()
    return nc


def _host_prep(x_q, x_kv, rotary_pos_emb, Wq, bq, Wk, bk, Wv, bv, Wp, bp):
    import ml_dtypes

    f = np.float32
    bfl = ml_dtypes.bfloat16
    x_q = np.asarray(x_q, f)
    x_kv = np.asarray(x_kv, f)
    freqs = np.asarray(rotary_pos_emb, f)

    # Even/odd pair-split permutation of the first 32 dims of each head, so
    # rotate_half becomes a 16-partition block swap on chip.
    perm = np.arange(C)
    for h in range(H):
        b0 = h * HD
        blk = np.empty(HD, np.int64)
        blk[0:16] = b0 + np.arange(0, 32, 2)
        blk[16:32] = b0 + np.arange(1, 32, 2)
        blk[32:64] = b0 + np.arange(32, 64)
        perm[b0:b0 + HD] = blk

    def wT(W, p=None):
        W = np.asarray(W, f)
        if p is not None:
            W = W[p, :]
        return np.ascontiguousarray(W.T).astype(bfl)

    cosE = np.cos(freqs[:, 0::2]).T  # [16, T]
    cosO = np.cos(freqs[:, 1::2]).T
    sinE = -np.sin(freqs[:, 0::2]).T
    sinO = np.sin(freqs[:, 1::2]).T
    cosP = np.ones((128, TP), f)
    sinP = np.zeros((128, TP), f)
    for s in (0, 64):
        cosP[s:s + 16, :T] = cosE
        cosP[s + 16:s + 32, :T] = cosO
        sinP[s:s + 16, :T] = sinE
        sinP[s + 16:s + 32, :T] = sinO


    p_idx = np.arange(128)[:, None]
    f_idx = np.arange(128)[None, :]
    m0 = (p_idx < f_idx).astype(f)

    def tile_qk(wt):  # [C, C] (ci, co) -> [p, co, ci, 128]
        return np.ascontiguousarray(
            wt.reshape(NCI, 128, NCO, 128).transpose(1, 2, 0, 3))

    def tile_vp(wt):  # [C, C] (ci, co) -> [p, ci, C]
        return np.ascontiguousarray(wt.reshape(NCI, 128, C).transpose(1, 0, 2))

    bqp = np.asarray(bq, f)[perm]
    bkp = np.asarray(bk, f)[perm]
    shared = {
        "wqT3": tile_qk(wT(Wq, perm)),
        "wkT3": tile_qk(wT(Wk, perm)),
        "wvT3": tile_vp(wT(Wv)),
        "wpT3": tile_vp(wT(Wp)),
        "bq2": np.ascontiguousarray(bqp.reshape(NCO, 128).T),
        "bk2": np.ascontiguousarray(bkp.reshape(NCO, 128).T),
        "bp2": np.ascontiguousarray(np.asarray(bp, f).reshape(NCO, 128).T),
        "bv1": np.asarray(bv, f).reshape(1, C).astype(bfl),
        "cosP": np.ascontiguousarray(cosP).astype(bfl),
        "sinP": np.ascontiguousarray(sinP).astype(bfl),
        "m0": np.ascontiguousarray(m0).astype(bfl),
    }

    def padT(xt):  # [C, T] -> [p, ci, TP]
        out = np.zeros((C, TP), f)
        out[:, :T] = xt
        return np.ascontiguousarray(
            out.reshape(NCI, 128, TP).transpose(1, 0, 2)).astype(bfl)

    in_maps = []
    for b in range(B):
        m = dict(shared)
        m["xqT3"] = padT(x_q[b].T)
        m["xkvT3"] = padT(x_kv[b].T)
        in_maps.append(m)
    return in_maps


def kernel(x_q, x_kv, rotary_pos_emb, Wq, bq, Wk, bk, Wv, bv, Wp, bp):
    from concourse.bass_utils import run_bass_kernel_spmd

    if "nc" not in _CACHE:
        _CACHE["nc"] = _build_program()
    nc = _CACHE["nc"]

    in_maps = _host_prep(x_q, x_kv, rotary_pos_emb,
                         Wq, bq, Wk, bk, Wv, bv, Wp, bp)
    trace = os.environ.get("BTK_TRACE", "0") == "1"
    res = run_bass_kernel_spmd(
        nc, in_maps, core_ids=list(range(B)), trace=trace)
    _CACHE["last_result"] = res
    return np.stack(
        [np.ascontiguousarray(r["outT"][:, :T].T).astype(np.float32)
         for r in res.results],
        axis=0)


# revision 102
# speedup vs baseline: 1.2496x; 1.0121x over previous
"""Trainium2 Bass kernel for CausalCrossAttention (B=8, T=769, C=1024, H=16).

Sharding: data-parallel over batch B=8 across the 8 NeuronCores (one batch
element per core, SPMD — identical program, different input slices).

v2 — all-bf16 matmul operands (fp32 PSUM accumulation), designed around the
trace of the fp32r v1 (340 us):
  * bf16 stationary operands get Fast Weight Load (fp32 LDWEIGHTS was ~45% of
    PE busy time in v1) and bf16 moving operands avoid the fp32r 4-cycle/row
    penalty on free dims < 256 (the attention boundary tiles).
  * S^T per head-pair via K=64 row-tiled matmul pairs (tile_position rows 0/64
    run concurrently in the PE array) -> one N-stream per pair instead of one
    per head, and no sibling-row zeroing.
  * The softmax exp chain on the scalar engine (~76 us serial) is the #2
    resource after the PE, so S^T+exp "doses" are interleaved into the Q/K/V
    projection emission to start it ~7 us in instead of after projections.
  * Output projection in [c_out, t] layout (host un-transposes) -> same cost
    as Q/K and no M=1 tail streams.
  * PV PSUM slots are released by plain copies into yT right after the PV
    matmuls; the softmax divide (denominator row -> DRAM -> partition
    broadcast -> reciprocal -> in-place mul) runs off the critical path.

PSUM budget: psP 4x[128,512] (projections, PV, out-proj) + psS 2x[128,1024]
(S^T tiles, exp-consumed) = 8 banks.
"""

import os

import numpy as np

B, T, C = 8, 769, 1024
H, HD, L = 16, 64, 32
COND = 256
NCI = 8  # 1024 / 128 contraction tiles
NCO = 8
NTT = 7  # t tiles: 6 full + 1 single row
TP = 770  # streamed T padded to even
VW = H * (HD + 1) + 63  # V_aug free width, padded so M=128 slices stay in-bounds

# Per-(kv-tile) q ranges in the 0:512 block + mask offset.
# nk covers kv cols [128*nk, 128*nk+128); allowed iff kv_col < 256 + q_col,
# i.e. p < f + 256 - 128*nk with p the in-tile kv index, f the abs q col.
R0SUB = {0: (0, None), 1: (0, None), 2: (0, 0),
         3: (128, 128), 4: (256, 256), 5: (384, 384)}

_CACHE = {}


def _build_program():
    import concourse.mybir as mybir
    import concourse.tile as tile
    from concourse import bacc

    f32 = mybir.dt.float32
    bf = mybir.dt.bfloat16
    Exp = mybir.ActivationFunctionType.Exp
    Ident = mybir.ActivationFunctionType.Identity
    Copy = mybir.ActivationFunctionType.Copy

    nc = bacc.Bacc("TRN2", target_bir_lowering=False)

    # All big inputs are host-pre-tiled to the SBUF layout [partition, free...]
    # so each DMA moves one large contiguous chunk per partition (the DMA
    # queues are descriptor-rate bound, ~1 descriptor per (partition, chunk)).
    xqT_d = nc.dram_tensor("xqT3", [128, NCI, TP], bf, kind="ExternalInput")
    xkvT_d = nc.dram_tensor("xkvT3", [128, NCI, TP], bf, kind="ExternalInput")
    # Q/K weights co-major [p, co, ci, 128] so the first co pair's slice is
    # one small leading DMA; V/P weights ci-major [p, ci, co].
    wq_d = nc.dram_tensor("wqT3", [128, NCO, NCI, 128], bf,
                          kind="ExternalInput")
    wk_d = nc.dram_tensor("wkT3", [128, NCO, NCI, 128], bf,
                          kind="ExternalInput")
    wv_d = nc.dram_tensor("wvT3", [128, NCI, C], bf, kind="ExternalInput")
    wp_d = nc.dram_tensor("wpT3", [128, NCI, C], bf, kind="ExternalInput")
    bq_d = nc.dram_tensor("bq2", [128, NCO], f32, kind="ExternalInput")
    bk_d = nc.dram_tensor("bk2", [128, NCO], f32, kind="ExternalInput")
    bp_d = nc.dram_tensor("bp2", [128, NCO], f32, kind="ExternalInput")
    bv_d = nc.dram_tensor("bv1", [1, C], bf, kind="ExternalInput")
    cos_d = nc.dram_tensor("cosP", [128, TP], bf, kind="ExternalInput")
    sin_d = nc.dram_tensor("sinP", [128, TP], bf, kind="ExternalInput")
    m0_d = nc.dram_tensor("m0", [128, 128], bf, kind="ExternalInput")
    outT_d = nc.dram_tensor("outT", [C, TP], bf, kind="ExternalOutput")

    with tile.TileContext(nc) as tc:
        with (
            tc.tile_pool(name="consts", bufs=1) as consts,
            tc.tile_pool(name="wpool", bufs=3) as wpool,
            tc.tile_pool(name="qkpool", bufs=1) as qkpool,
            tc.tile_pool(name="vpool", bufs=1) as vpool,
            tc.tile_pool(name="shpool", bufs=2) as shpool,
            tc.tile_pool(name="ptpool", bufs=3) as ptpool,
            tc.tile_pool(name="ypool", bufs=1) as ypool,
            tc.tile_pool(name="stgpool", bufs=2) as stgpool,
            tc.tile_pool(name="rdbcpool", bufs=2) as rdbcpool,
            tc.tile_pool(name="psP", bufs=2, space="PSUM") as psP,
            tc.tile_pool(name="psS", bufs=2, space="PSUM") as psS,
            tc.tile_pool(name="dramp", bufs=1, space="DRAM") as dramp,
        ):
            # ---- constants ----
            # cos/sin duplicated along a co-pair axis so rotary runs on
            # [128, 2, TP] blocks (two c_out tiles at once).
            cos1 = consts.tile([128, TP], bf, tag="cos1")
            sin1 = consts.tile([128, TP], bf, tag="sin1")
            nc.scalar.dma_start(out=cos1, in_=cos_d[:, :])
            nc.scalar.dma_start(out=sin1, in_=sin_d[:, :])
            m0_sb = consts.tile([128, 128], bf, tag="m0")
            nc.scalar.dma_start(out=m0_sb, in_=m0_d[:, :])
            bq_sb = consts.tile([128, NCO], f32, tag="bq")
            bk_sb = consts.tile([128, NCO], f32, tag="bk")
            bp_sb = consts.tile([128, NCO], f32, tag="bp")
            nc.scalar.dma_start(out=bq_sb, in_=bq_d[:, :])
            nc.scalar.dma_start(out=bk_sb, in_=bk_d[:, :])
            nc.scalar.dma_start(out=bp_sb, in_=bp_d[:, :])
            ones16 = consts.tile([128, 16], bf, tag="ones16")
            nc.vector.memset(ones16, 1.0)
            zcol = consts.tile([128, 128], bf, tag="zcol")
            nc.vector.memset(zcol, 0.0)
            bv_sb = consts.tile([128, C], bf, tag="bv")

            dnd = dramp.tile([H, TP], f32, tag="dnd")

            # ---- persistent activations ----
            qT = qkpool.tile([128, NCI, TP], bf, tag="qT")
            kT = qkpool.tile([128, NCI, TP], bf, tag="kT")
            vaug = vpool.tile([128, NTT, VW], bf, tag="vaug")



            # ---- S^T + exp dose machinery ----
            # Each dose is ~0.2 us of PE work gated on one psS slot pair; the
            # pump() calls sprinkle them through the projection emission at
            # roughly the scalar engine's exp drain rate.
            st_queue = []
            pair_pts = [dict() for _ in range(NCI)]
            pair_done = [False] * NCI

            def pump(n=1):
                # Emit all available doses eagerly — the Tile scheduler
                # reorders by readiness, so a dose waiting on an exp slot
                # doesn't block projection matmuls behind it.
                while st_queue:
                    st_queue.pop(0)()

            def ensure_pair(j):
                while not pair_done[j]:
                    st_queue.pop(0)()

            def dose_nk(j, nk):
                # pt tiles hold local q coords [qlo:770) -> [0:770-qlo) so the
                # pool slots are exactly sized (deeper pipeline per KB).
                qlo, moff = R0SUB[nk]
                w = TP - qlo
                sta = psS.tile([128, 1024], f32, tag="st", name=f"sta{j}_{nk}")
                stb = psS.tile([128, 1024], f32, tag="st", name=f"stb{j}_{nk}")
                ks = slice(nk * 128, (nk + 1) * 128)
                nc.tensor.matmul(sta[:, qlo:512], kT[0:64, j, ks],
                                 qT[0:64, j, qlo:512], start=True, stop=True)
                nc.tensor.matmul(sta[:, 512:770], kT[0:64, j, ks],
                                 qT[0:64, j, 512:770], start=True, stop=True)
                nc.tensor.matmul(stb[:, qlo:512], kT[64:128, j, ks],
                                 qT[64:128, j, qlo:512], start=True, stop=True)
                nc.tensor.matmul(stb[:, 512:770], kT[64:128, j, ks],
                                 qT[64:128, j, 512:770], start=True, stop=True)
                pa = ptpool.tile([128, w], bf, tag=f"pta{nk}",
                                 name=f"pta{j}_{nk}")
                pb = ptpool.tile([128, w], bf, tag=f"ptb{nk}",
                                 name=f"ptb{j}_{nk}")
                nc.scalar.activation(out=pa[:, 0:w], in_=sta[:, qlo:770],
                                     func=Exp, scale=0.125)
                nc.scalar.activation(out=pb[:, 0:w], in_=stb[:, qlo:770],
                                     func=Exp, scale=0.125)
                if moff is not None:
                    nc.gpsimd.tensor_mul(pa[:, 0:128], pa[:, 0:128], m0_sb)
                    nc.gpsimd.tensor_mul(pb[:, 0:128], pb[:, 0:128], m0_sb)
                pair_pts[j]["a", nk] = pa
                pair_pts[j]["b", nk] = pb

            def dose_tail(j):
                # kv col 768 (single kv row); q col 512 is masked -> zeroed.
                st6a = psS.tile([128, 1024], f32, tag="st", name=f"st6a{j}")
                st6b = psS.tile([128, 1024], f32, tag="st", name=f"st6b{j}")
                nc.tensor.matmul(st6a[0:1, 0:258], kT[0:64, j, 768:769],
                                 qT[0:64, j, 512:770], start=True, stop=True)
                nc.tensor.matmul(st6b[0:1, 0:258], kT[64:128, j, 768:769],
                                 qT[64:128, j, 512:770], start=True, stop=True)
                p6a = ptpool.tile([1, 258], bf, tag="pt6a", name=f"pt6a{j}")
                p6b = ptpool.tile([1, 258], bf, tag="pt6b", name=f"pt6b{j}")
                nc.scalar.activation(out=p6a[0:1, 1:258],
                                     in_=st6a[0:1, 1:258], func=Exp,
                                     scale=0.125)
                nc.scalar.activation(out=p6b[0:1, 1:258],
                                     in_=st6b[0:1, 1:258], func=Exp,
                                     scale=0.125)
                nc.vector.tensor_copy(p6a[0:1, 0:1], zcol[0:1, 0:1])
                nc.vector.tensor_copy(p6b[0:1, 0:1], zcol[0:1, 0:1])
                pair_pts[j]["a", 6] = p6a
                pair_pts[j]["b", 6] = p6b
                pair_done[j] = True

            def push_pair(j):
                for nk in range(6):
                    st_queue.append(lambda j=j, nk=nk: dose_nk(j, nk))
                st_queue.append(lambda j=j: dose_tail(j))

            # ---- Q/K projection (in [c_out, t] layout) + rotary ----
            def emit_qk_proj(w, x, b_sb, outT_t, co, nm):
                ps = psP.tile([128, 1024], f32, tag="ps", name=f"{nm}P{co}")
                for ci in range(NCI):
                    lhs = w[:, co, ci, :]
                    nc.tensor.matmul(ps[:, 0:512], lhs, x[:, ci, 0:512],
                                     start=(ci == 0), stop=(ci == NCI - 1))
                    nc.tensor.matmul(ps[:, 512:770], lhs, x[:, ci, 512:770],
                                     start=(ci == 0), stop=(ci == NCI - 1))
                    if ci in (2, 5):
                        pump()
                nc.vector.tensor_scalar_add(
                    outT_t[:, co, :], ps[:, 0:770], b_sb[:, co:co + 1])

            def emit_rot(outT_t, co, nm):
                # partial rotary on the (host-permuted) first 32 dims of each
                # head: 16-row partition-block swap + q*cos + swapped*sin.
                # cos rows outside the rotary dims are 1.0, sin rows 0.0.
                # sh swaps ride the scalar queue, which drains its share of
                # the input burst early — they never sit behind multi-MB
                # weight transfers there.
                # rows 32:64 of sh skip the swap DMA (sin there is 0.0); a
                # cheap memset keeps them finite for the 0:96-row multiply.
                q = nc.sync if nm == "q" else nc.gpsimd
                blk = outT_t[:, co, :]
                sh = shpool.tile([128, TP], bf, tag="sh", name=f"sh{nm}{co}")
                for s in (0, 64):
                    q.dma_start(out=sh[s:s + 16], in_=blk[s + 16:s + 32])
                    q.dma_start(out=sh[s + 16:s + 32], in_=blk[s:s + 16])
                nc.vector.memset(sh[32:64], 0.0)
                nc.vector.tensor_mul(sh[0:96], sh[0:96], sin1[0:96])
                nc.vector.tensor_mul(blk, blk, cos1)
                nc.vector.tensor_add(blk[0:96], blk[0:96], sh[0:96])

            # xq/xkv are dead after the V projection; scoping them lets the
            # yT / out-staging pools reuse the same SBUF region.
            xpool_cm = tc.tile_pool(name="xpool", bufs=1)
            xpool = xpool_cm.__enter__()
            xq = xpool.tile([128, NCI, TP], bf, tag="xq")
            xkv = xpool.tile([128, NCI, TP], bf, tag="xkv")
            # Startup burst spread across all five engine DMA queues (each is
            # ~85 GB/s): x and Q/K weights land by ~18 us so the exp chain can
            # start ~30 us in. Waiting triggers (wp reuses wq's pool slot)
            # stay off the tensor/vector/gpsimd queues — a waiting trigger
            # head-blocks everything behind it in that queue.
            # Only sync/scalar/gpsimd engines can trigger DMAs (~85 GB/s per
            # queue). Upfront: just the wq/wk co 0-1 slices, the x tensors
            # (1-ci chunks round-robined over all three queues) and consts —
            # everything else trickles in per-co inside the loop so the
            # latency-critical rotary sh swaps never queue behind multi-MB
            # transfers. The kernel is paced by the exp chain (~10 us/pair),
            # so later weights have ample queue time.
            wq = wpool.tile([128, NCO, NCI, 128], bf, tag="w", name="wq")
            wk = wpool.tile([128, NCO, NCI, 128], bf, tag="w", name="wk")
            wv = wpool.tile([128, NCI, C], bf, tag="w", name="wv")
            nc.sync.dma_start(out=wq[:, 0:2], in_=wq_d[:, 0:2])
            nc.gpsimd.dma_start(out=wk[:, 0:2], in_=wk_d[:, 0:2])
            # all xq chunks land first (Q co0 needs the full tensor), xkv
            # behind them — Q co0 starts ~13 us in, K co0 ~20 us.
            qs3 = (nc.sync, nc.gpsimd, nc.scalar)
            for ci in range(NCI):
                qs3[ci % 3].dma_start(out=xq[:, ci], in_=xqT_d[:, ci])
            for ci in range(NCI):
                qs3[ci % 3].dma_start(out=xkv[:, ci], in_=xkvT_d[:, ci])
            # ---- V projection (in [t, c_out] layout, ones-augmented) ----
            def emit_v_group(tg):
                tiles = {}
                for tt in tg:
                    tiles[tt] = psP.tile([128, 1024], f32, tag="ps",
                                         name=f"v{tt}")
                for ci in range(NCI):
                    for tt in tg:
                        tsz = 128 if tt < 6 else 1
                        lhs = xkv[:, ci, tt * 128:tt * 128 + tsz]
                        for hf in (0, 1):
                            nc.tensor.matmul(
                                tiles[tt][:tsz, hf * 512:(hf + 1) * 512], lhs,
                                wv[:, ci, hf * 512:(hf + 1) * 512],
                                start=(ci == 0), stop=(ci == NCI - 1))
                for tt in tg:
                    tsz = 128 if tt < 6 else 1
                    va = vaug[:tsz, tt, 0:H * (HD + 1)].rearrange(
                        "p (h e) -> p h e", e=HD + 1)
                    nc.vector.tensor_add(
                        va[:, :, 0:HD],
                        tiles[tt][:tsz, :].rearrange("p (h d) -> p h d", h=H),
                        bv_sb[:tsz, :].rearrange("p (h d) -> p h d", h=H))
                    nc.vector.tensor_copy(
                        va[:, :, HD:HD + 1], ones16[:tsz, :].unsqueeze(2))
                    # pad tail so M=128 lhsT slices stay initialized
                    nc.vector.tensor_copy(
                        vaug[:tsz, tt, H * (HD + 1):VW],
                        zcol[:tsz, 0:VW - H * (HD + 1)])

            yT = ypool.tile([128, NCI, TP], bf, tag="yT")

            # ---- attention PV + softmax divide ----
            def emit_pv(j, side):
                h = 2 * j + (0 if side == "a" else 1)
                vs = slice(h * (HD + 1), h * (HD + 1) + 128)
                pts = pair_pts[j]
                o = psP.tile([128, 1024], f32, tag="ps", name=f"pv{h}")
                for nk in range(6):
                    qlo, _ = R0SUB[nk]
                    p = pts[side, nk]
                    nc.tensor.matmul(o[:, qlo:512], vaug[:, nk, vs],
                                     p[:, 0:512 - qlo], start=(nk == 0),
                                     stop=False)
                    nc.tensor.matmul(o[:, 512:770], vaug[:, nk, vs],
                                     p[:, 512 - qlo:TP - qlo],
                                     start=(nk == 0), stop=False)
                p6 = pts[side, 6]
                nc.tensor.matmul(o[:, 512:770], vaug[0:1, 6, vs],
                                 p6[0:1, 0:258], start=False, stop=True)
                return o

            def emit_div(j, side, o, stg):
                # release the PSUM slot quickly: a plain copy into yT plus an
                # ACT/DVE-copy of the denominator row to SBUF staging (DMA
                # cannot read PSUM); the divide happens in-place on yT once
                # the broadcast lands.
                h = 2 * j + (0 if side == "a" else 1)
                r = slice(0, 64) if side == "a" else slice(64, 128)
                srow = 0 if side == "a" else 1
                nc.vector.tensor_copy(yT[r, j, :], o[0:64, 0:770])
                if side == "a":  # split staging between ACT and DVE so the
                    # exp chain on ACT only absorbs half of it
                    nc.scalar.activation(out=stg[0:1, srow, :],
                                         in_=o[HD:HD + 1, 0:770], func=Copy)
                else:
                    nc.vector.tensor_copy(stg[0:1, srow, :],
                                          o[HD:HD + 1, 0:770])
                nc.gpsimd.dma_start(out=dnd[h:h + 1, :], in_=stg[0:1, srow, :])

            def emit_pv_pair(j):
                ensure_pair(j)
                stg = stgpool.tile([1, 2, TP], f32, tag="stg", name=f"stg{j}")
                oa = emit_pv(j, "a")
                emit_div(j, "a", oa, stg)
                ob = emit_pv(j, "b")
                emit_div(j, "b", ob, stg)
                pump()
                # denominator rows broadcast across partitions via DRAM, one
                # reciprocal for the pair, then the in-place divide of yT.
                rdbc = rdbcpool.tile([128, TP], f32, tag="rdbc",
                                     name=f"rdbc{j}")
                nc.gpsimd.dma_start(
                    out=rdbc[0:64, :],
                    in_=dnd[2 * j:2 * j + 1, :].broadcast_to((64, TP)))
                nc.gpsimd.dma_start(
                    out=rdbc[64:128, :],
                    in_=dnd[2 * j + 1:2 * j + 2, :].broadcast_to((64, TP)))
                nc.vector.reciprocal_approx_fast(out=rdbc, in_=rdbc)
                nc.vector.tensor_mul(yT[0:64, j, :], yT[0:64, j, :],
                                     rdbc[0:64, :])
                nc.vector.tensor_mul(yT[64:128, j, :], yT[64:128, j, :],
                                     rdbc[64:128, :])

            # V tt-groups go at co 2-5 (wv trickles in during co 0-1) so vaug
            # completes mid-loop, and PV pairs 0-3 interleave at co 4-7 —
            # freeing pt slots so pairs 4-7's exps never wait on PV.
            v_at = {2: [(0, 1)], 3: [(2, 3)], 4: [(4, 5), (6,)]}
            pv_at = {4: 0, 5: 1, 6: 2, 7: 3}
            for co in range(NCO):
                emit_qk_proj(wq, xq, bq_sb, qT, co, "q")
                emit_rot(qT, co, "q")
                if co < NCO - 2:  # trickle the remaining weight columns
                    nc.sync.dma_start(out=wq[:, co + 2], in_=wq_d[:, co + 2])
                pump()
                emit_qk_proj(wk, xkv, bk_sb, kT, co, "k")
                emit_rot(kT, co, "k")
                if co < NCO - 2:
                    nc.gpsimd.dma_start(out=wk[:, co + 2], in_=wk_d[:, co + 2])
                if co < 2:  # wv fully needed by the first V group (co 2)
                    for i in range(4):
                        q = nc.sync if i % 2 == 0 else nc.gpsimd
                        q.dma_start(out=wv[:, 4 * co + i],
                                    in_=wv_d[:, 4 * co + i])
                if co == 1:
                    nc.gpsimd.dma_start(
                        out=bv_sb, in_=bv_d[0:1, :].broadcast_to((128, C)))
                push_pair(co)
                pump()
                for g in v_at.get(co, ()):
                    emit_v_group(g)
                if co in pv_at:
                    emit_pv_pair(pv_at[co])
            # wp reuses wq's slot; its triggers wait for the Q projection's
            # last weight read, so they are emitted after everything else
            # that shares the sync queue in this phase.
            wp = wpool.tile([128, NCI, C], bf, tag="w", name="wp")
            nc.sync.dma_start(out=wp[:, 0:4], in_=wp_d[:, 0:4])
            nc.sync.dma_start(out=wp[:, 4:NCI], in_=wp_d[:, 4:NCI])
            xpool_cm.__exit__(None, None, None)
            otpool_cm = tc.tile_pool(name="otpool", bufs=2)
            otpool = otpool_cm.__enter__()

            for j in range(4, NCI):
                emit_pv_pair(j)

            # ---- output projection (in [c_out, t] layout) ----
            # bf16 output (+0.4% rounding, well within tolerance) and the
            # store DMAs rotate over all three queues so the endgame isn't
            # bound by one ~85 GB/s queue.
            for co in range(NCO):
                ps = psP.tile([128, 1024], f32, tag="ps", name=f"oP{co}")
                for ci in range(NCI):
                    lhs = wp[:, ci, co * 128:(co + 1) * 128]
                    nc.tensor.matmul(ps[:, 0:512], lhs, yT[:, ci, 0:512],
                                     start=(ci == 0), stop=(ci == NCI - 1))
                    nc.tensor.matmul(ps[:, 512:770], lhs, yT[:, ci, 512:770],
                                     start=(ci == 0), stop=(ci == NCI - 1))
                ot = otpool.tile([128, TP], bf, tag="ot", name=f"ot{co}")
                nc.scalar.activation(out=ot[:, :], in_=ps[:, 0:770],
                                     func=Ident, bias=bp_sb[:, co:co + 1],
                                     scale=1.0)
                qs3[co % 3].dma_start(
                    out=outT_d[co * 128:(co + 1) * 128, :], in_=ot[:, :])

            otpool_cm.__exit__(None, None, None)

    nc.compile()
    return nc


def _host_prep(x_q, x_kv, rotary_pos_emb, Wq, bq, Wk, bk, Wv, bv, Wp, bp):
    import ml_dtypes

    f = np.float32
    bfl = ml_dtypes.bfloat16
    x_q = np.asarray(x_q, f)
    x_kv = np.asarray(x_kv, f)
    freqs = np.asarray(rotary_pos_emb, f)

    # Even/odd pair-split permutation of the first 32 dims of each head, so
    # rotate_half becomes a 16-partition block swap on chip.
    perm = np.arange(C)
    for h in range(H):
        b0 = h * HD
        blk = np.empty(HD, np.int64)
        blk[0:16] = b0 + np.arange(0, 32, 2)
        blk[16:32] = b0 + np.arange(1, 32, 2)
        blk[32:64] = b0 + np.arange(32, 64)
        perm[b0:b0 + HD] = blk

    def wT(W, p=None):
        W = np.asarray(W, f)
        if p is not None:
            W = W[p, :]
        return np.ascontiguousarray(W.T).astype(bfl)

    cosE = np.cos(freqs[:, 0::2]).T  # [16, T]
    cosO = np.cos(freqs[:, 1::2]).T
    sinE = -np.sin(freqs[:, 0::2]).T
    sinO = np.sin(freqs[:, 1::2]).T
    cosP = np.ones((128, TP), f)
    sinP = np.zeros((128, TP), f)
    for s in (0, 64):
        cosP[s:s + 16, :T] = cosE
        cosP[s + 16:s + 32, :T] = cosO
        sinP[s:s + 16, :T] = sinE
        sinP[s + 16:s + 32, :T] = sinO


    p_idx = np.arange(128)[:, None]
    f_idx = np.arange(128)[None, :]
    m0 = (p_idx < f_idx).astype(f)

    def tile_qk(wt):  # [C, C] (ci, co) -> [p, co, ci, 128]
        return np.ascontiguousarray(
            wt.reshape(NCI, 128, NCO, 128).transpose(1, 2, 0, 3))

    def tile_vp(wt):  # [C, C] (ci, co) -> [p, ci, C]
        return np.ascontiguousarray(wt.reshape(NCI, 128, C).transpose(1, 0, 2))

    bqp = np.asarray(bq, f)[perm]
    bkp = np.asarray(bk, f)[perm]
    shared = {
        "wqT3": tile_qk(wT(Wq, perm)),
        "wkT3": tile_qk(wT(Wk, perm)),
        "wvT3": tile_vp(wT(Wv)),
        "wpT3": tile_vp(wT(Wp)),
        "bq2": np.ascontiguousarray(bqp.reshape(NCO, 128).T),
        "bk2": np.ascontiguousarray(bkp.reshape(NCO, 128).T),
        "bp2": np.ascontiguousarray(np.asarray(bp, f).reshape(NCO, 128).T),
        "bv1": np.asarray(bv, f).reshape(1, C).astype(bfl),
        "cosP": np.ascontiguousarray(cosP).astype(bfl),
        "sinP": np.ascontiguousarray(sinP).astype(bfl),
        "m0": np.ascontiguousarray(m0).astype(bfl),
    }

    def padT(xt):  # [C, T] -> [p, ci, TP]
        out = np.zeros((C, TP), f)
        out[:, :T] = xt
        return np.ascontiguousarray(
            out.reshape(NCI, 128, TP).transpose(1, 0, 2)).astype(bfl)

    in_maps = []
    for b in range(B):
        m = dict(shared)
        m["xqT3"] = padT(x_q[b].T)
        m["xkvT3"] = padT(x_kv[b].T)
        in_maps.append(m)
    return in_maps


def kernel(x_q, x_kv, rotary_pos_emb, Wq, bq, Wk, bk, Wv, bv, Wp, bp):
    from concourse.bass_utils import run_bass_kernel_spmd

    if "nc" not in _CACHE:
        _CACHE["nc"] = _build_program()
    nc = _CACHE["nc"]

    in_maps = _host_prep(x_q, x_kv, rotary_pos_emb,
                         Wq, bq, Wk, bk, Wv, bv, Wp, bp)
    trace = os.environ.get("BTK_TRACE", "0") == "1"
    res = run_bass_kernel_spmd(
        nc, in_maps, core_ids=list(range(B)), trace=trace)
    _CACHE["last_result"] = res
    return np.stack(
        [np.ascontiguousarray(r["outT"][:, :T].T).astype(np.float32)
         for r in res.results],
        axis=0)
